# Initial kernel scaffold
#
"""Trainium2 Bass kernel for nn_BasicSelection: per-mesh edge-MLP + face gather/mean.

Reference computation (per mesh b of 8):
    h  = x[b].T                      # [E, 64]
    fe = sigmoid(mlp(h))             # [E, 1]  (64->128->128->64->1, ReLU hidden)
    out[b, f] = mean(fe[etof[b, f, k]] for k in 0..2)

Sharding: pure data parallelism — mesh b on NeuronCore b (B == 8 == n_cores).

Per-core dataflow:
  - Features live on SBUF partitions, edges on the free dim. Supertile = 1024
    edges = two 512-edge halves (A at partitions 0-63 of the x tile, B at
    64-127) so layer-1 (K=64) and layer-3 (M=64) run as packed concurrent
    matmul pairs via tile_position, and layer-4 (M=1) packs 4 outputs per
    PSUM bank across col groups.
  - Matmul operands are bf16 (fp32 LOW_HIGH double-pass mode never leaves
    the 1.2 GHz throttled clock on this part); PSUM accumulation stays fp32,
    so the end-to-end error is ~2e-4. Layers are software-pipelined across
    supertiles (layer k of supertile i-k per iteration) to keep the PE dense.
  - PSUM drains fuse bias+ReLU (DVE tensor_scalar / ACT activation) and
    bias+sigmoid for the head; fe is written densely to a fat DRAM scratch.
  - Gather+mean: 100 SWDGE indirect DMAs (3072 one-element descriptors each,
    spread over the 4 qPoolDynamic queues so their random-read drains
    overlap) gather the 3F face-edge values; DVE sums the 3 slot groups and
    scales by 1/3. Offsets are host-pre-permuted for the partition-minor
    SWDGE consumption order and the fat fe layout.
"""

import numpy as np

import concourse.bacc as bacc
import concourse.bass as bass
import concourse.tile as tile
import concourse.mybir as mybir
from concourse.bass_utils import run_bass_kernel_spmd

B, NIN, E, F = 8, 64, 150000, 100000
ST = 1024                 # edges per supertile
NST = 148                 # supertiles (even, 148*1024 >= E)
EP = NST * ST             # padded edge count: 151552
# Gather geometry: 16 indirect-DMA instructions, each generating
# GND = 128*GCOLS single-element descriptors into one SBUF partition row.
GQ = 100                  # gather instructions / gout partitions
GCOLS = 24                # offset-tile columns per instruction (128*24 descs
                          # per instruction, under the ~16K SWDGE ring cap)
GND = 128 * GCOLS         # descriptors per instruction (18816)
GF = GND // 3             # faces per gout partition (6272)
NIDX = GQ * GCOLS         # offset tile free dim (2352)
FPAD = GQ * GF            # padded face count (100352)

f32 = mybir.dt.float32
bf16 = mybir.dt.bfloat16
i32 = mybir.dt.int32
Alu = mybir.AluOpType
Act = mybir.ActivationFunctionType

# Number of fe chunks gathered in separate passes so gather DMA overlaps the
# MLP. Chunks are aligned to the 2-supertile (2048 edge) fe write granularity.
GATHER_CHUNKS = 1


def _chunk_bounds():
    groups = NST // 2  # fe is written in 2048-edge groups
    per = (groups + GATHER_CHUNKS - 1) // GATHER_CHUNKS
    bounds = []
    lo = 0
    for _ in range(GATHER_CHUNKS):
        hi = min(lo + per * 2048, EP)
        bounds.append((lo, hi))
        lo = hi
    return bounds


def build_nc():
    nc = bacc.Bacc(None, target_bir_lowering=False, num_swdge_queues=4)
    x_d = nc.dram_tensor('x', [NST, 128, 512], f32, kind='ExternalInput')
    etof_d = nc.dram_tensor('etof', [128, NIDX], i32, kind='ExternalInput')
    w0_d = nc.dram_tensor('w0', [128, 128], bf16, kind='ExternalInput')
    b0_d = nc.dram_tensor('b0', [128, 1], f32, kind='ExternalInput')
    w1_d = nc.dram_tensor('w1', [128, 128], bf16, kind='ExternalInput')
    b1_d = nc.dram_tensor('b1', [128, 1], f32, kind='ExternalInput')
    w2_d = nc.dram_tensor('w2', [128, 64], bf16, kind='ExternalInput')
    b2_d = nc.dram_tensor('b2', [128, 1], f32, kind='ExternalInput')
    w3_d = nc.dram_tensor('w3', [128, 32], bf16, kind='ExternalInput')
    b3_d = nc.dram_tensor('b3', [128, 1], f32, kind='ExternalInput')
    out_d = nc.dram_tensor('out', [GQ, GF], f32, kind='ExternalOutput')

    # fe scratch in "fat" layout: one dense [128, 512] tile per 2-supertile
    # group (rows 0/32/64/96 hold real fe; the rest is garbage). Dense writes
    # keep the DMA descriptor/semaphore accounting trivial; the gather offsets
    # are host-transformed into this layout.
    fe_fat_d = nc.dram_tensor('fefat', [NST // 2, 128, 512], f32, kind='Internal')

    with tile.TileContext(nc) as tc:
        with (
            tc.tile_pool(name='wpool', bufs=1) as wp,
            tc.tile_pool(name='xpool', bufs=3) as xp,
            tc.tile_pool(name='hpool', bufs=2) as hp,
            tc.tile_pool(name='gpool', bufs=1) as gp,
            tc.tile_pool(name='psum', bufs=1, space='PSUM') as pp,
            tc.tile_pool(name='psum1', bufs=2, space='PSUM') as pp1,
            tc.tile_pool(name='psum3', bufs=1, space='PSUM') as pp3,
        ):
            w0_t = wp.tile([128, 128], bf16, tag='w0')
            w1_t = wp.tile([128, 128], bf16, tag='w1')
            w2_t = wp.tile([128, 64], bf16, tag='w2')
            w3_t = wp.tile([128, 32], bf16, tag='w3')
            b0_t = wp.tile([128, 1], f32, tag='b0')
            b1_t = wp.tile([128, 1], f32, tag='b1')
            b2_t = wp.tile([128, 1], f32, tag='b2')
            b3_t = wp.tile([128, 1], f32, tag='b3')
            for t, d in [(w0_t, w0_d), (w1_t, w1_d), (w2_t, w2_d), (w3_t, w3_d),
                         (b0_t, b0_d), (b1_t, b1_d), (b2_t, b2_d), (b3_t, b3_d)]:
                nc.sync.dma_start(t[:], d[:])

            idx_t = gp.tile([128, NIDX], i32, tag='idx')
            nc.sync.dma_start(idx_t[:], etof_d[:])
            gout = gp.tile([GQ, GND], f32, tag='gout')

            def r(ap):
                # matmul operands are bf16: fp32 LOW_HIGH double-pass mode
                # never leaves the 1.2 GHz cold clock on this part, so bf16
                # halves pass count and runs warm.
                return ap

            # Software pipeline: iteration i runs layer 1 of supertile i,
            # layer 2 of i-1, layer 3 of i-2, layer 4 of i-3 — so the PE never
            # waits on the current supertile's PSUM drain and stays warm.
            p1s = {}
            p2s = {}
            p3s = {}
            h1s = {}
            h2s = {}
            h3s = {}
            p4 = None
            for i in range(NST + 3):
                s1, s2, s3, s4 = i, i - 1, i - 2, i - 3
                if s1 < NST:
                    xts = xp.tile([128, 512], f32, tag='xts')
                    nc.sync.dma_start(xts[:], x_d[s1])
                    xt = xp.tile([128, 512], bf16, tag='xt')
                    nc.vector.tensor_copy(xt[:], xts[:])
                    p1 = pp1.tile([128, 1024], f32, tag='p1')
                    p1s[s1] = p1
                    nc.tensor.matmul(p1[:, 0:512], r(w0_t[0:64, :]),
                                     r(xt[0:64, :]), tile_position=(0, 0))
                    nc.tensor.matmul(p1[:, 512:1024], r(w0_t[64:128, :]),
                                     r(xt[64:128, :]), tile_position=(64, 0))
                    h1 = hp.tile([128, 1024], bf16, tag='h1')
                    h1s[s1] = h1
                    nc.vector.tensor_scalar(h1[:], p1[:], b0_t[:, 0:1], 0.0,
                                            Alu.add, Alu.max)
                if 0 <= s2 < NST:
                    h1 = h1s.pop(s2)
                    p2 = pp.tile([128, 1024], f32, tag='p2')
                    p2s[s2] = p2
                    nc.tensor.matmul(p2[:, 0:512], r(w1_t[:]),
                                     r(h1[:, 0:512]))
                    nc.tensor.matmul(p2[:, 512:1024], r(w1_t[:]),
                                     r(h1[:, 512:1024]))
                    h2 = hp.tile([128, 1024], bf16, tag='h2')
                    h2s[s2] = h2
                    nc.scalar.activation(h2[:], p2[:], Act.Relu,
                                         bias=b1_t[:, 0:1])
                if 0 <= s3 < NST:
                    h2 = h2s.pop(s3)
                    p3 = pp3.tile([128, 512], f32, tag='p3')
                    p3s[s3] = p3
                    nc.tensor.matmul(p3[0:64, :], r(w2_t[:]),
                                     r(h2[:, 0:512]), tile_position=(0, 0))
                    nc.tensor.matmul(p3[64:128, :], r(w2_t[:]),
                                     r(h2[:, 512:1024]), tile_position=(0, 64))
                    h3 = hp.tile([128, 512], bf16, tag='h3')
                    h3s[s3] = h3
                    if s3 % 2 == 0:
                        nc.vector.tensor_scalar(h3[:], p3[:], b2_t[:, 0:1],
                                                0.0, Alu.add, Alu.max)
                    else:
                        nc.scalar.activation(h3[:], p3[:], Act.Relu,
                                             bias=b2_t[:, 0:1])
                if 0 <= s4 < NST:
                    h3 = h3s.pop(s4)
                    if s4 % 2 == 0:
                        p4 = pp.tile([128, 512], f32, tag='p4')
                    cg = (s4 % 2) * 64
                    nc.tensor.matmul(p4[cg:cg + 32, :], r(w3_t[0:64, :]),
                                     r(h3[0:64, :]), tile_position=(0, cg))
                    nc.tensor.matmul(p4[cg + 32:cg + 64, :],
                                     r(w3_t[64:128, :]), r(h3[64:128, :]),
                                     tile_position=(64, cg + 32))
                    if s4 % 2 == 1:
                        fes = hp.tile([128, 512], f32, tag='fes')
                        nc.scalar.activation(fes[:], p4[:], Act.Sigmoid,
                                             bias=b3_t[:, 0:1])
                        nc.sync.dma_start(fe_fat_d[(s4 - 1) // 2, :, :], fes[:])

            # Gather: GQ indirect DMAs, each writing one gout partition row.
            # Descriptor k of instruction q uses offset element
            # idx_t[k % 128, q*GCOLS + k // 128] (SWDGE consumes the offset
            # tile partition-minor); the host pre-permutes etof accordingly.
            fe_dram = fe_fat_d[:].rearrange('g p e -> (g p e)').unsqueeze(-1)
            for q in range(GQ):
                gi = nc.gpsimd.indirect_dma_start(
                    out=gout[q:q + 1, :].unsqueeze(-1),
                    out_offset=None,
                    in_=fe_dram,
                    in_offset=bass.IndirectOffsetOnAxis(
                        ap=idx_t[:, q * GCOLS:(q + 1) * GCOLS], axis=0),
                )
                # spread across the 4 SWDGE queues so the random-read drains
                # of consecutive gathers overlap instead of serializing on
                # one descriptor ring
                gi.queue = f'qPoolDynamic{q % 4 or ""}'

            # Mean over the 3 vertical slot groups, then scale by 1/3.
            res = gp.tile([GQ, GF], f32, tag='res')
            nc.vector.tensor_tensor(res[:], gout[:, 0:GF], gout[:, GF:2 * GF],
                                    Alu.add)
            nc.vector.tensor_tensor(res[:], res[:], gout[:, 2 * GF:3 * GF],
                                    Alu.add)
            nc.vector.tensor_scalar_mul(res[:], res[:], 1.0 / 3.0)
            nc.sync.dma_start(out_d[:], res[:])

    nc.compile()
    return nc


def _bf(a):
    import ml_dtypes
    return np.ascontiguousarray(a.astype(ml_dtypes.bfloat16))


def _prep_core_inputs(x_b, etof_b, W0, b0, W1, b1, W2, b2, W3, b3):
    xp = np.zeros((NIN, EP), dtype=np.float32)
    xp[:, :E] = x_b
    # supertile-contiguous layout: x_dev[s, 64*h + f, e] = x[f, s*1024 + 512h + e]
    x_dev = np.ascontiguousarray(
        xp.reshape(NIN, NST, 2, 512).transpose(1, 2, 0, 3).reshape(NST, 128, 512))
    et = np.zeros((FPAD, 3), dtype=np.int64)
    et[:F] = etof_b
    # edge e lives at fe_fat[e >> 11, 32 * ((e >> 9) & 3), e & 511]
    et = ((et >> 11) << 16) | (((et >> 9) & 3) << 14) | (et & 511)
    # The mean is symmetric over a face's 3 slots, so sort each face's three
    # addresses into the slots, then assign faces to gather slots in
    # min-address order: the slot-0 read stream is fully address-sorted and
    # slots 1/2 are partially sorted (DRAM row locality). The host inverts
    # the face permutation on the output.
    et = np.sort(et, axis=1)
    order = np.argsort(et[:, 0], kind='stable').astype(np.int64)
    et = et[order]
    # gout[q, n] <- fe[idx_dev[n % 128, q*GCOLS + n // 128]]; we want
    # gout[q, g + k*GF] = fe[etof[q*GF + g, k]].
    p, c = np.mgrid[0:128, 0:NIDX]
    q = c // GCOLS
    n = (c % GCOLS) * 128 + p
    g = n % GF
    k = n // GF
    et_dev = np.ascontiguousarray(et[q * GF + g, k]).astype(np.int32)
    return order, {
        'x': x_dev,
        'etof': et_dev,
        'w0': _bf(np.concatenate([W0, W0], axis=0)),
        'b0': np.ascontiguousarray(b0[:, None]),
        'w1': _bf(W1),
        'b1': np.ascontiguousarray(b1[:, None]),
        'w2': _bf(W2),
        'b2': np.ascontiguousarray(np.concatenate([b2, b2], axis=0)[:, None]),
        'w3': _bf(np.tile(np.concatenate([W3, W3], axis=0), (1, 32))),
        'b3': np.full((128, 1), b3[0], dtype=np.float32),
    }


_NC = None


def _get_nc():
    global _NC
    if _NC is None:
        _NC = build_nc()
    return _NC


def kernel(x, etof, W0, b0, W1, b1, W2, b2, W3, b3, _trace=False):
    x = np.asarray(x, dtype=np.float32)
    etof = np.asarray(etof, dtype=np.int32)
    args = [np.asarray(a, dtype=np.float32)
            for a in (W0, b0, W1, b1, W2, b2, W3, b3)]
    nc = _get_nc()
    prepped = [_prep_core_inputs(x[b], etof[b], *args) for b in range(B)]
    orders = [p[0] for p in prepped]
    in_maps = [p[1] for p in prepped]
    r = run_bass_kernel_spmd(nc, in_maps, core_ids=list(range(B)), trace=_trace)
    out = np.empty((B, F, 1), dtype=np.float32)
    for b in range(B):
        full = np.empty(FPAD, dtype=np.float32)
        full[orders[b]] = r.results[b]['out'].reshape(-1)
        out[b, :, 0] = full[:F]
    if _trace:
        return out, r
    return out



# revision 1
# speedup vs baseline: 1.1633x; 1.1633x over previous
"""Trainium2 Bass kernel for nn_BasicSelection: per-mesh edge-MLP + face gather/mean.

Reference computation (per mesh b of 8):
    h  = x[b].T                      # [E, 64]
    fe = sigmoid(mlp(h))             # [E, 1]  (64->128->128->64->1, ReLU hidden)
    out[b, f] = mean(fe[etof[b, f, k]] for k in 0..2)

Sharding: pure data parallelism — mesh b on NeuronCore b (B == 8 == n_cores).

Per-core dataflow:
  - Features live on SBUF partitions, edges on the free dim. Supertile = 1024
    edges = two 512-edge halves (A at partitions 0-63 of the x tile, B at
    64-127) so layer-1 (K=64) and layer-3 (M=64) run as packed concurrent
    matmul pairs via tile_position, and layer-4 (M=1) packs 4 outputs per
    PSUM bank across col groups.
  - Matmul operands are bf16 (fp32 LOW_HIGH double-pass mode never leaves
    the 1.2 GHz throttled clock on this part); PSUM accumulation stays fp32,
    so the end-to-end error is ~2e-4. Layers are software-pipelined across
    supertiles (layer k of supertile i-k per iteration) to keep the PE dense.
  - PSUM drains fuse bias+ReLU (DVE tensor_scalar / ACT activation) and
    bias+sigmoid for the head; fe is written densely to a fat DRAM scratch.
  - Gather+mean: 100 SWDGE indirect DMAs (3072 one-element descriptors each,
    spread over the 4 qPoolDynamic queues so their random-read drains
    overlap) gather the 3F face-edge values; DVE sums the 3 slot groups and
    scales by 1/3. Offsets are host-pre-permuted for the partition-minor
    SWDGE consumption order and the fat fe layout.
"""

import numpy as np

import concourse.bacc as bacc
import concourse.bass as bass
import concourse.tile as tile
import concourse.mybir as mybir
from concourse.bass_utils import run_bass_kernel_spmd

B, NIN, E, F = 8, 64, 150000, 100000
ST = 1024                 # edges per supertile
NST = 148                 # supertiles (even, 148*1024 >= E)
EP = NST * ST             # padded edge count: 151552
# Gather geometry: 16 indirect-DMA instructions, each generating
# GND = 128*GCOLS single-element descriptors into one SBUF partition row.
GQ = 100                  # gather instructions / gout partitions
GCOLS = 24                # offset-tile columns per instruction (128*24 descs
                          # per instruction, under the ~16K SWDGE ring cap)
GND = 128 * GCOLS         # descriptors per instruction (18816)
GF = GND // 3             # faces per gout partition (6272)
NIDX = GQ * GCOLS         # offset tile free dim (2352)
FPAD = GQ * GF            # padded face count (100352)

f32 = mybir.dt.float32
bf16 = mybir.dt.bfloat16
i32 = mybir.dt.int32
Alu = mybir.AluOpType
Act = mybir.ActivationFunctionType

# Number of fe chunks gathered in separate passes so gather DMA overlaps the
# MLP. Chunks are aligned to the 2-supertile (2048 edge) fe write granularity.
GATHER_CHUNKS = 1


def _chunk_bounds():
    groups = NST // 2  # fe is written in 2048-edge groups
    per = (groups + GATHER_CHUNKS - 1) // GATHER_CHUNKS
    bounds = []
    lo = 0
    for _ in range(GATHER_CHUNKS):
        hi = min(lo + per * 2048, EP)
        bounds.append((lo, hi))
        lo = hi
    return bounds


def build_nc():
    nc = bacc.Bacc(None, target_bir_lowering=False, num_swdge_queues=4)
    x_d = nc.dram_tensor('x', [NST, 128, 512], f32, kind='ExternalInput')
    etof_d = nc.dram_tensor('etof', [128, NIDX], i32, kind='ExternalInput')
    w0_d = nc.dram_tensor('w0', [128, 128], bf16, kind='ExternalInput')
    b0_d = nc.dram_tensor('b0', [128, 1], f32, kind='ExternalInput')
    w1_d = nc.dram_tensor('w1', [128, 128], bf16, kind='ExternalInput')
    b1_d = nc.dram_tensor('b1', [128, 1], f32, kind='ExternalInput')
    w2_d = nc.dram_tensor('w2', [128, 64], bf16, kind='ExternalInput')
    b2_d = nc.dram_tensor('b2', [128, 1], f32, kind='ExternalInput')
    w3_d = nc.dram_tensor('w3', [128, 32], bf16, kind='ExternalInput')
    b3_d = nc.dram_tensor('b3', [128, 1], f32, kind='ExternalInput')
    out_d = nc.dram_tensor('out', [GQ, GF], f32, kind='ExternalOutput')

    # fe scratch in "fat" layout: one dense [128, 512] tile per 2-supertile
    # group (rows 0/32/64/96 hold real fe; the rest is garbage). Dense writes
    # keep the DMA descriptor/semaphore accounting trivial; the gather offsets
    # are host-transformed into this layout.
    fe_fat_d = nc.dram_tensor('fefat', [NST // 2, 128, 512], f32, kind='Internal')

    with tile.TileContext(nc) as tc:
        with (
            tc.tile_pool(name='wpool', bufs=1) as wp,
            tc.tile_pool(name='xpool', bufs=3) as xp,
            tc.tile_pool(name='hpool', bufs=2) as hp,
            tc.tile_pool(name='gpool', bufs=1) as gp,
            tc.tile_pool(name='psum', bufs=1, space='PSUM') as pp,
            tc.tile_pool(name='psum1', bufs=2, space='PSUM') as pp1,
            tc.tile_pool(name='psum3', bufs=1, space='PSUM') as pp3,
        ):
            w0_t = wp.tile([128, 128], bf16, tag='w0')
            w1_t = wp.tile([128, 128], bf16, tag='w1')
            w2_t = wp.tile([128, 64], bf16, tag='w2')
            w3_t = wp.tile([128, 32], bf16, tag='w3')
            b0_t = wp.tile([128, 1], f32, tag='b0')
            b1_t = wp.tile([128, 1], f32, tag='b1')
            b2_t = wp.tile([128, 1], f32, tag='b2')
            b3_t = wp.tile([128, 1], f32, tag='b3')
            for t, d in [(w0_t, w0_d), (w1_t, w1_d), (w2_t, w2_d), (w3_t, w3_d),
                         (b0_t, b0_d), (b1_t, b1_d), (b2_t, b2_d), (b3_t, b3_d)]:
                nc.sync.dma_start(t[:], d[:])

            idx_t = gp.tile([128, NIDX], i32, tag='idx')
            nc.sync.dma_start(idx_t[:], etof_d[:])
            gout = gp.tile([GQ, GND], f32, tag='gout')

            def r(ap):
                # matmul operands are bf16: fp32 LOW_HIGH double-pass mode
                # never leaves the 1.2 GHz cold clock on this part, so bf16
                # halves pass count and runs warm.
                return ap

            # Software pipeline: iteration i runs layer 1 of supertile i,
            # layer 2 of i-1, layer 3 of i-2, layer 4 of i-3 — so the PE never
            # waits on the current supertile's PSUM drain and stays warm.
            p1s = {}
            p2s = {}
            p3s = {}
            h1s = {}
            h2s = {}
            h3s = {}
            p4 = None
            for i in range(NST + 3):
                s1, s2, s3, s4 = i, i - 1, i - 2, i - 3
                if s1 < NST:
                    xts = xp.tile([128, 512], f32, tag='xts')
                    nc.sync.dma_start(xts[:], x_d[s1])
                    xt = xp.tile([128, 512], bf16, tag='xt')
                    nc.vector.tensor_copy(xt[:], xts[:])
                    p1 = pp1.tile([128, 1024], f32, tag='p1')
                    p1s[s1] = p1
                    nc.tensor.matmul(p1[:, 0:512], r(w0_t[0:64, :]),
                                     r(xt[0:64, :]), tile_position=(0, 0))
                    nc.tensor.matmul(p1[:, 512:1024], r(w0_t[64:128, :]),
                                     r(xt[64:128, :]), tile_position=(64, 0))
                    h1 = hp.tile([128, 1024], bf16, tag='h1')
                    h1s[s1] = h1
                    nc.vector.tensor_scalar(h1[:], p1[:], b0_t[:, 0:1], 0.0,
                                            Alu.add, Alu.max)
                if 0 <= s2 < NST:
                    h1 = h1s.pop(s2)
                    p2 = pp.tile([128, 1024], f32, tag='p2')
                    p2s[s2] = p2
                    nc.tensor.matmul(p2[:, 0:512], r(w1_t[:]),
                                     r(h1[:, 0:512]))
                    nc.tensor.matmul(p2[:, 512:1024], r(w1_t[:]),
                                     r(h1[:, 512:1024]))
                    h2 = hp.tile([128, 1024], bf16, tag='h2')
                    h2s[s2] = h2
                    nc.scalar.activation(h2[:], p2[:], Act.Relu,
                                         bias=b1_t[:, 0:1])
                if 0 <= s3 < NST:
                    h2 = h2s.pop(s3)
                    p3 = pp3.tile([128, 512], f32, tag='p3')
                    p3s[s3] = p3
                    nc.tensor.matmul(p3[0:64, :], r(w2_t[:]),
                                     r(h2[:, 0:512]), tile_position=(0, 0))
                    nc.tensor.matmul(p3[64:128, :], r(w2_t[:]),
                                     r(h2[:, 512:1024]), tile_position=(0, 64))
                    h3 = hp.tile([128, 512], bf16, tag='h3')
                    h3s[s3] = h3
                    if s3 % 2 == 0:
                        nc.vector.tensor_scalar(h3[:], p3[:], b2_t[:, 0:1],
                                                0.0, Alu.add, Alu.max)
                    else:
                        nc.scalar.activation(h3[:], p3[:], Act.Relu,
                                             bias=b2_t[:, 0:1])
                if 0 <= s4 < NST:
                    h3 = h3s.pop(s4)
                    if s4 % 2 == 0:
                        p4 = pp.tile([128, 512], f32, tag='p4')
                    cg = (s4 % 2) * 64
                    nc.tensor.matmul(p4[cg:cg + 32, :], r(w3_t[0:64, :]),
                                     r(h3[0:64, :]), tile_position=(0, cg))
                    nc.tensor.matmul(p4[cg + 32:cg + 64, :],
                                     r(w3_t[64:128, :]), r(h3[64:128, :]),
                                     tile_position=(64, cg + 32))
                    if s4 % 2 == 1:
                        fes = hp.tile([128, 512], f32, tag='fes')
                        nc.scalar.activation(fes[:], p4[:], Act.Sigmoid,
                                             bias=b3_t[:, 0:1])
                        nc.sync.dma_start(fe_fat_d[(s4 - 1) // 2, :, :], fes[:])

            # Gather: GQ indirect DMAs, each writing one gout partition row.
            # Descriptor k of instruction q uses offset element
            # idx_t[k % 128, q*GCOLS + k // 128] (SWDGE consumes the offset
            # tile partition-minor); the host pre-permutes etof accordingly.
            fe_dram = fe_fat_d[:].rearrange('g p e -> (g p e)').unsqueeze(-1)
            for q in range(GQ):
                gi = nc.gpsimd.indirect_dma_start(
                    out=gout[q:q + 1, :].unsqueeze(-1),
                    out_offset=None,
                    in_=fe_dram,
                    in_offset=bass.IndirectOffsetOnAxis(
                        ap=idx_t[:, q * GCOLS:(q + 1) * GCOLS], axis=0),
                )
                # spread across the 4 SWDGE queues so the random-read drains
                # of consecutive gathers overlap instead of serializing on
                # one descriptor ring
                gi.queue = f'qPoolDynamic{q % 4 or ""}'

            # Mean over the 3 vertical slot groups, then scale by 1/3.
            res = gp.tile([GQ, GF], f32, tag='res')
            nc.vector.tensor_tensor(res[:], gout[:, 0:GF], gout[:, GF:2 * GF],
                                    Alu.add)
            nc.vector.tensor_tensor(res[:], res[:], gout[:, 2 * GF:3 * GF],
                                    Alu.add)
            nc.vector.tensor_scalar_mul(res[:], res[:], 1.0 / 3.0)
            nc.sync.dma_start(out_d[:], res[:])

    nc.compile()
    return nc


def _bf(a):
    import ml_dtypes
    return np.ascontiguousarray(a.astype(ml_dtypes.bfloat16))


def _prep_core_inputs(x_b, etof_b, W0, b0, W1, b1, W2, b2, W3, b3):
    xp = np.zeros((NIN, EP), dtype=np.float32)
    xp[:, :E] = x_b
    # supertile-contiguous layout: x_dev[s, 64*h + f, e] = x[f, s*1024 + 512h + e]
    x_dev = np.ascontiguousarray(
        xp.reshape(NIN, NST, 2, 512).transpose(1, 2, 0, 3).reshape(NST, 128, 512))
    et = np.zeros((FPAD, 3), dtype=np.int64)
    et[:F] = etof_b
    # edge e lives at fe_fat[e >> 11, 32 * ((e >> 9) & 3), e & 511]
    et = ((et >> 11) << 16) | (((et >> 9) & 3) << 14) | (et & 511)
    # The mean is symmetric over a face's 3 slots, so sort each face's three
    # addresses into the slots, then assign faces to gather slots in
    # min-address order: the slot-0 read stream is fully address-sorted and
    # slots 1/2 are partially sorted (DRAM row locality). The host inverts
    # the face permutation on the output.
    et = np.sort(et, axis=1)
    order = np.argsort(et[:, 0], kind='stable').astype(np.int64)
    et = et[order]
    # gout[q, n] <- fe[idx_dev[n % 128, q*GCOLS + n // 128]]; we want
    # gout[q, g + k*GF] = fe[etof[q*GF + g, k]].
    p, c = np.mgrid[0:128, 0:NIDX]
    q = c // GCOLS
    n = (c % GCOLS) * 128 + p
    g = n % GF
    k = n // GF
    et_dev = np.ascontiguousarray(et[q * GF + g, k]).astype(np.int32)
    return order, {
        'x': x_dev,
        'etof': et_dev,
        'w0': _bf(np.concatenate([W0, W0], axis=0)),
        'b0': np.ascontiguousarray(b0[:, None]),
        'w1': _bf(W1),
        'b1': np.ascontiguousarray(b1[:, None]),
        'w2': _bf(W2),
        'b2': np.ascontiguousarray(np.concatenate([b2, b2], axis=0)[:, None]),
        'w3': _bf(np.tile(np.concatenate([W3, W3], axis=0), (1, 32))),
        'b3': np.full((128, 1), b3[0], dtype=np.float32),
    }


_NC = None


def _get_nc():
    global _NC
    if _NC is None:
        _NC = build_nc()
    return _NC


def kernel(x, etof, W0, b0, W1, b1, W2, b2, W3, b3, _trace=False):
    x = np.asarray(x, dtype=np.float32)
    etof = np.asarray(etof, dtype=np.int32)
    args = [np.asarray(a, dtype=np.float32)
            for a in (W0, b0, W1, b1, W2, b2, W3, b3)]
    nc = _get_nc()
    prepped = [_prep_core_inputs(x[b], etof[b], *args) for b in range(B)]
    orders = [p[0] for p in prepped]
    in_maps = [p[1] for p in prepped]
    r = run_bass_kernel_spmd(nc, in_maps, core_ids=list(range(B)), trace=_trace)
    out = np.empty((B, F, 1), dtype=np.float32)
    for b in range(B):
        full = np.empty(FPAD, dtype=np.float32)
        full[orders[b]] = r.results[b]['out'].reshape(-1)
        out[b, :, 0] = full[:F]
    if _trace:
        return out, r
    return out

